# revision 4
# baseline (speedup 1.0000x reference)
"""Trainium2 Bass kernel for top-2 MoE routing (nn_JaxMoE_26431228740246).

Strategy: expert parallel across 8 NeuronCores (1 expert per core) with
SPARSE dispatch.  The reference computes every expert densely over all 2048
tokens, but only the top-2 experts per token carry nonzero combine weight, so
each core only needs its expert's assigned tokens (~512 avg, 551 max here).

Host side (the dispatch/combine layer of expert parallelism): router
softmax + top-2 + renormalize in fp32 numpy, gather each expert's tokens
into a fixed-capacity [D, C] transposed slab (C = 576 covers the max load
with margin), scatter-add the weighted expert outputs back to [T, D].

Device side (per core): plain SwiGLU MLP over C tokens in bf16 —
h = silu(x@Wg) * (x@Wu); out = h @ Wd — streaming weights from DRAM,
fp32 PSUM accumulation, output [D, C] fp32.  No router, no transpose, no
collectives on device.

Shapes (hardcoded): T=2048, D=1024, F=4096, E=8, K=2.
"""

import os
import sys

import numpy as np
import ml_dtypes


def _ensure_path():
    for p in (
        "/root/.axon_site",
        "/root/.axon_site/_ro/trn_rl_repo",
        "/root/.axon_site/_ro/pypackages",
        "/opt/trn_rl_repo",
    ):
        if os.path.isdir(p) and p not in sys.path:
            sys.path.append(p)


_ensure_path()

T, D, F, E = 2048, 1024, 4096, 8
DT = D // 128       # 8 d-tiles
FTILES = F // 128   # 32 f-tiles
FC = 512            # f columns per gate/up weight DMA chunk
DC = 256            # d columns per down-proj weight DMA chunk

_CACHE = {}


def _chunks(C):
    out, c0 = [], 0
    while c0 < C:
        cw = min(512, C - c0)
        out.append((c0, cw))
        c0 += cw
    return out


def _build(C):
    import concourse.tile as tile
    from concourse import bacc, mybir

    fp32 = mybir.dt.float32
    bf16 = mybir.dt.bfloat16
    Act = mybir.ActivationFunctionType

    chunks = _chunks(C)

    nc = bacc.Bacc("TRN2", target_bir_lowering=False, debug=False, num_devices=E)

    xT = nc.dram_tensor("xT", [D, C], bf16, kind="ExternalInput").ap()
    wg = nc.dram_tensor("wg", [D, F], bf16, kind="ExternalInput").ap()
    wu = nc.dram_tensor("wu", [D, F], bf16, kind="ExternalInput").ap()
    wd = nc.dram_tensor("wd", [F, D], bf16, kind="ExternalInput").ap()
    outT = nc.dram_tensor("outT", [D, C], fp32, kind="ExternalOutput").ap()

    xT_r = xT.rearrange("(do di) c -> di do c", di=128)      # [128, 8, C]
    wg_r = wg.rearrange("(do di) f -> di do f", di=128)      # [128, 8, F]
    wu_r = wu.rearrange("(do di) f -> di do f", di=128)
    wd_r = wd.rearrange("(fo fi) d -> fi fo d", fi=128)      # [128, 32, D]

    from contextlib import ExitStack

    with tile.TileContext(nc) as tc, ExitStack() as ctx:
        pconst = ctx.enter_context(tc.tile_pool(name="const", bufs=1))
        ph = ctx.enter_context(tc.tile_pool(name="h", bufs=1))
        pwgu = ctx.enter_context(tc.tile_pool(name="wgu", bufs=2))
        pwd = ctx.enter_context(tc.tile_pool(name="wd", bufs=2))
        posb = ctx.enter_context(tc.tile_pool(name="osb", bufs=2))
        ptmp = ctx.enter_context(tc.tile_pool(name="tmp", bufs=2))
        pmm = {
            cw: ctx.enter_context(
                tc.tile_pool(name=f"mm{cw}", bufs=4, space="PSUM")
            )
            for cw in sorted({cw for _, cw in chunks})
        }

        xsb = pconst.tile([128, DT, C], bf16, tag="xsb")
        nc.sync.dma_start(xsb[:], xT_r[:])

        h = ph.tile([128, FTILES, C], bf16, tag="h")

        # ---- gate/up -> h ----
        for fc in range(F // FC):
            wg_t = pwgu.tile([128, DT, FC], bf16, tag="wg")
            nc.sync.dma_start(wg_t[:], wg_r[:, :, fc * FC : (fc + 1) * FC])
            wu_t = pwgu.tile([128, DT, FC], bf16, tag="wu")
            nc.sync.dma_start(wu_t[:], wu_r[:, :, fc * FC : (fc + 1) * FC])
            for fi in range(FC // 128):
                k = fc * (FC // 128) + fi
                for c0, cw in chunks:
                    pg = pmm[cw].tile([128, cw], fp32, tag=f"mm{cw}")
                    for do in range(DT):
                        nc.tensor.matmul(
                            pg[:],
                            wg_t[:, do, fi * 128 : (fi + 1) * 128],
                            xsb[:, do, c0 : c0 + cw],
                            start=(do == 0),
                            stop=(do == DT - 1),
                        )
                    tmp = ptmp.tile([128, cw], fp32, tag=f"tmp{cw}")
                    nc.scalar.activation(tmp[:], pg[:], Act.Silu)
                    pu = pmm[cw].tile([128, cw], fp32, tag=f"mm{cw}")
                    for do in range(DT):
                        nc.tensor.matmul(
                            pu[:],
                            wu_t[:, do, fi * 128 : (fi + 1) * 128],
                            xsb[:, do, c0 : c0 + cw],
                            start=(do == 0),
                            stop=(do == DT - 1),
                        )
                    nc.vector.tensor_mul(h[:, k, c0 : c0 + cw], tmp[:], pu[:])

        # ---- down-projection ----
        for dp in range(D // DC):
            wd_t = pwd.tile([128, FTILES, DC], bf16, tag="wd")
            nc.sync.dma_start(wd_t[:], wd_r[:, :, dp * DC : (dp + 1) * DC])
            for di in range(DC // 128):
                dd = dp * (DC // 128) + di
                for c0, cw in chunks:
                    po = pmm[cw].tile([128, cw], fp32, tag=f"mm{cw}")
                    for k in range(FTILES):
                        nc.tensor.matmul(
                            po[:],
                            wd_t[:, k, di * 128 : (di + 1) * 128],
                            h[:, k, c0 : c0 + cw],
                            start=(k == 0),
                            stop=(k == FTILES - 1),
                        )
                    osb = posb.tile([128, cw], fp32, tag=f"osb{cw}")
                    nc.vector.tensor_copy(osb[:], po[:])
                    nc.sync.dma_start(
                        outT[dd * 128 : (dd + 1) * 128, c0 : c0 + cw], osb[:]
                    )

    nc.compile()
    return nc


def _get_nc(C):
    key = ("nc", C)
    if key not in _CACHE:
        _CACHE[key] = _build(C)
    return _CACHE[key]


def _bf16(a):
    return np.ascontiguousarray(np.asarray(a, dtype=np.float32)).astype(
        ml_dtypes.bfloat16
    )


def kernel(
    x_TD, w_router_DE, kernel_gating_EDF, kernel_up_proj_EDF, kernel_down_proj_EFD
):
    from concourse.bass_utils import run_bass_kernel_spmd

    x = np.ascontiguousarray(np.asarray(x_TD, dtype=np.float32))
    wr = np.ascontiguousarray(np.asarray(w_router_DE, dtype=np.float32))
    g = np.asarray(kernel_gating_EDF, dtype=np.float32)
    u = np.asarray(kernel_up_proj_EDF, dtype=np.float32)
    d = np.asarray(kernel_down_proj_EFD, dtype=np.float32)

    # ---- router (fp32, exact top-2 + renormalize) ----
    logits = x @ wr
    p = np.exp(logits - logits.max(axis=-1, keepdims=True))
    p /= p.sum(axis=-1, keepdims=True)
    rows = np.arange(T)
    i1 = p.argmax(axis=-1)
    p2 = p.copy()
    p2[rows, i1] = -1.0
    i2 = p2.argmax(axis=-1)
    v1, v2 = p[rows, i1], p[rows, i2]
    s = v1 + v2
    w1, w2 = v1 / s, v2 / s

    idxs, wts = [], []
    for e in range(E):
        m1 = i1 == e
        sel = m1 | (i2 == e)
        idx = np.nonzero(sel)[0]
        idxs.append(idx)
        wts.append(np.where(m1, w1, w2)[idx].astype(np.float32))

    L = max(len(ix) for ix in idxs)
    C = max(576, -(-L // 64) * 64)  # capacity; 576 covers the seed-0 loads
    nc = _get_nc(C)

    xT = _bf16(x.T)  # [D, T]
    in_maps = []
    for e in range(E):
        xTe = np.zeros((D, C), dtype=ml_dtypes.bfloat16)
        xTe[:, : len(idxs[e])] = xT[:, idxs[e]]
        in_maps.append(
            {"xT": xTe, "wg": _bf16(g[e]), "wu": _bf16(u[e]), "wd": _bf16(d[e])}
        )

    trace = bool(os.environ.get("BASS_PROF"))
    try:
        res = run_bass_kernel_spmd(nc, in_maps, list(range(E)), trace=trace)
    except Exception:
        if not trace:
            raise
        res = run_bass_kernel_spmd(nc, in_maps, list(range(E)), trace=False)
    _CACHE["last_result"] = res

    out = np.zeros((T, D), dtype=np.float32)
    for e in range(E):
        ye = np.asarray(res.results[e]["outT"], dtype=np.float32)
        out[idxs[e]] += wts[e][:, None] * ye[:, : len(idxs[e])].T
    return out
